# revision 3
# baseline (speedup 1.0000x reference)
"""GPT-2 style transformer block on 8 TRN2 NeuronCores.

Sharding: token-data-parallel. Each batch's 2048 tokens are split into 8
chunks of 256; core c owns batch c//4 and chunks {j, 7-j} (j = c%4) so
causal attention work is balanced. QKV/proj/MLP/LN are purely local; the
only collectives are two AllGathers (k^T+v combined, in two halves)
within each 4-core batch group, bf16 on the wire so the gathered data is
DMA'd straight into matmul layouts (no cast pass).

Score matmuls are head-PAIR packed: heads 2p/2p+1 run as two concurrent
row-tiled K=64 matmuls (tile_position row groups 0/64) writing the two
banks of one [128,1024] PSUM tile; one exp covers both heads, and the
causal 0/1 mask is applied with a stride-0-broadcast tensor_tensor so a
single 512-wide mask slab serves both heads. proj is pair-packed too
(K=128 = two heads' hd rows). The v-bias is folded into the proj bias
via softmax-rows-sum-to-one; the attention 1/sqrt(hd) scale is folded
into w_q. Softmax runs without max-subtraction (scores are O(1));
denominators come free as a 65th ones-column in the av matmul.
"""

import os
import sys

sys.path.insert(0, "/opt/trn_rl_repo")

import numpy as np
import ml_dtypes

import concourse.bass as bass
import concourse.tile as tile
from concourse import bacc, mybir
from concourse.bass_utils import run_bass_kernel_spmd
from concourse.masks import make_identity

F32 = mybir.dt.float32
BF16 = mybir.dt.bfloat16
BF = ml_dtypes.bfloat16

B, T, C, H, HD = 2, 2048, 768, 12, 64
EPS = 1e-5
NCORES = 8
CHUNK = 256            # global chunk size (tokens)
TLOC = 512             # local tokens per core (2 chunks)
NKT = T // 128         # 16 key tiles per batch
MASK_W = 8 * 512 + 8 * 256   # 6144

KH2 = 3 * 128 * TLOC   # k^T half: [3 ct][128 p][512 t]
VH2 = 4 * 128 * 384    # v half:   [4 tt][128 p][384 c]
HC = 384

# exp groups per head-pair: kt<8 -> one kt per [128,1024] psum
# (both heads x 512); kt>=8 -> two kt per psum (both heads x 2x256).
GROUPS2 = [(kt,) for kt in range(8)] + [(8, 9), (10, 11), (12, 13), (14, 15)]

LAST_EXEC_NS = None
LAST_RESULTS = None
_CACHE = {}


def _build(add_qk_bias, add_proj_bias, add_fc2_bias):
    nc = bacc.Bacc("TRN2", target_bir_lowering=False, debug=False,
                   num_devices=NCORES)

    x_ext = nc.dram_tensor("x", [TLOC, C], F32, kind="ExternalInput")
    wq_ext = nc.dram_tensor("wq", [C, C], BF16, kind="ExternalInput")
    wk_ext = nc.dram_tensor("wk", [C, C], BF16, kind="ExternalInput")
    wv_ext = nc.dram_tensor("wv", [C, C], BF16, kind="ExternalInput")
    wp_ext = nc.dram_tensor("wp", [6, 128, C], BF16, kind="ExternalInput")
    wfc_ext = nc.dram_tensor("wfc", [C, 4 * C], BF16, kind="ExternalInput")
    wfc2_ext = nc.dram_tensor("wfc2", [4 * C, C], BF16, kind="ExternalInput")
    masks_ext = nc.dram_tensor("masks", [128, MASK_W], BF16,
                               kind="ExternalInput")
    bqk_ext = nc.dram_tensor("bqk", [2, C], F32, kind="ExternalInput")
    bfc_ext = nc.dram_tensor("bfc", [4 * C], F32, kind="ExternalInput")
    bout_ext = nc.dram_tensor("bout", [2, C], F32, kind="ExternalInput")
    out_ext = nc.dram_tensor("out", [TLOC, C], F32, kind="ExternalOutput")

    with tile.TileContext(nc) as tc:
        with tc.tile_pool(name="dram", bufs=1, space="DRAM") as dram, \
             tc.tile_pool(name="singles", bufs=1) as singles, \
             tc.tile_pool(name="persist", bufs=1) as persist, \
             tc.tile_pool(name="small", bufs=3) as small:

            kv_in1 = dram.tile([KH2 + VH2], BF16)
            kv_all1 = dram.tile([4, KH2 + VH2], BF16)
            kv_in2 = dram.tile([KH2 + VH2], BF16)
            kv_all2 = dram.tile([4, KH2 + VH2], BF16)

            # x first: LN1 needs it before anything else
            x_sb = persist.tile([128, 4, C], F32)     # local x, becomes xmid
            for t in range(4):
                nc.sync.dma_start(out=x_sb[:, t, :],
                                  in_=x_ext[t * 128:(t + 1) * 128, :])

            ident = singles.tile([128, 128], BF16)
            make_identity(nc, ident)
            eps_sb = singles.tile([128, 1], F32)
            nc.vector.memset(eps_sb, EPS)
            ones_pad = singles.tile([128, 64], F32)
            nc.vector.memset(ones_pad, 0.0)
            nc.vector.memset(ones_pad[0:1, :], 1.0)
            d_sb = singles.tile([128, 2, TLOC], F32)
            nc.vector.memset(d_sb, 1.0)

            bqk_sb = singles.tile([128, 2, 6], F32)
            if add_qk_bias:
                nc.sync.dma_start(
                    out=bqk_sb,
                    in_=bqk_ext.ap().rearrange("b (m p) -> p b m", p=128))
            bout_sb = singles.tile([128, 2, C], F32)
            if add_proj_bias or add_fc2_bias:
                bc = bout_ext.ap()
                nc.sync.dma_start(
                    out=bout_sb,
                    in_=bass.AP(tensor=bc.tensor, offset=bc.offset,
                                ap=[[0, 128], bc.ap[0], bc.ap[1]]))

            masks_sb = persist.tile([128, MASK_W], BF16)
            hT = persist.tile([128, 6, TLOC], BF16)   # h^T, reused for h2^T
            qT = persist.tile([128, 6, TLOC], BF16)   # head-pair layout
            yT = persist.tile([128, 6, TLOC], BF16)   # head-pair layout
            wp_sb = persist.tile([128, 6, C], BF16)
            bfc_sb = singles.tile([128, 24], F32)

            def layernorm_to(pool, xt, dst, tagsuf):
                stats = pool.tile([128, 3, 6], F32, tag="st" + tagsuf,
                                  name="st" + tagsuf)
                for sg in range(3):
                    nc.vector.bn_stats(out=stats[:, sg, :],
                                       in_=xt[:, sg * 256:(sg + 1) * 256])
                mv = pool.tile([128, 2], F32, tag="mv" + tagsuf,
                               name="mv" + tagsuf)
                nc.vector.bn_aggr(out=mv, in_=stats)
                nc.scalar.activation(out=mv[:, 1:2], in_=mv[:, 1:2],
                                     func=mybir.ActivationFunctionType.Sqrt,
                                     bias=eps_sb)
                nc.vector.reciprocal(out=mv[:, 1:2], in_=mv[:, 1:2])
                nc.vector.tensor_scalar(out=dst, in0=xt,
                                        scalar1=mv[:, 0:1], scalar2=mv[:, 1:2],
                                        op0=mybir.AluOpType.subtract,
                                        op1=mybir.AluOpType.mult)

            # ---------------- LN1 + transpose + QKV + AGs ----------------
            with tc.tile_pool(name="ln", bufs=3) as lnp, \
                 tc.tile_pool(name="tp", bufs=2, space="PSUM") as tpp, \
                 tc.tile_pool(name="qkp", bufs=2, space="PSUM") as qkp, \
                 tc.tile_pool(name="vp", bufs=2, space="PSUM") as vpp, \
                 tc.tile_pool(name="vsb", bufs=1) as vsbp:

                kT = vsbp.tile([128, 6, TLOC], BF16)
                v_sb1 = vsbp.tile([128, 4, HC], BF16)
                v_sb2 = vsbp.tile([128, 4, HC], BF16)
                wk_sb = vsbp.tile([128, 6, C], BF16)
                wq_sb = vsbp.tile([128, 6, C], BF16)
                wv_sb = vsbp.tile([128, 6, C], BF16)
                for sb, ext in ((wk_sb, wk_ext), (wv_sb, wv_ext),
                                (wq_sb, wq_ext)):
                    nc.sync.dma_start(
                        out=sb,
                        in_=ext.ap().rearrange("(ct p) c -> p ct c", p=128))

                for t in range(4):
                    xn = lnp.tile([128, C], BF16, tag="xn")
                    layernorm_to(lnp, x_sb[:, t, :], xn, "1")
                    for ct in range(6):
                        pt = tpp.tile([128, 128], BF16, tag="tp")
                        nc.tensor.transpose(
                            pt, xn[:, ct * 128:(ct + 1) * 128], ident)
                        nc.scalar.copy(hT[:, ct, t * 128:(t + 1) * 128], pt)

                def k_mtile(m):
                    ps = qkp.tile([128, TLOC], F32, tag="qk", name="psk")
                    for k in range(6):
                        nc.tensor.matmul(
                            ps, lhsT=wk_sb[:, k, m * 128:(m + 1) * 128],
                            rhs=hT[:, k, :], start=(k == 0), stop=(k == 5))
                    if add_qk_bias:
                        nc.scalar.activation(
                            out=kT[:, m, :], in_=ps,
                            func=mybir.ActivationFunctionType.Copy,
                            bias=bqk_sb[:, 1, m:m + 1])
                    else:
                        nc.scalar.copy(kT[:, m, :], ps)

                def v_half(half, v_sb):
                    c0 = half * HC
                    for tt in range(4):
                        pv = vpp.tile([128, HC], F32, tag="v")
                        for k in range(6):
                            nc.tensor.matmul(
                                pv, lhsT=hT[:, k, tt * 128:(tt + 1) * 128],
                                rhs=wv_sb[:, k, c0:c0 + HC],
                                start=(k == 0), stop=(k == 5))
                        nc.scalar.copy(v_sb[:, tt, :], pv)

                # half 1: k m=0..2 + v cols 0:384 -> AG1
                for m in range(3):
                    k_mtile(m)
                nc.sync.dma_start(
                    out=kv_in1[0:KH2].rearrange("(ct p t) -> p ct t", p=128,
                                                t=TLOC),
                    in_=kT[:, 0:3, :])
                v_half(0, v_sb1)
                nc.sync.dma_start(
                    out=kv_in1[KH2:KH2 + VH2].rearrange(
                        "(tt p c) -> p tt c", p=128, c=HC),
                    in_=v_sb1)
                nc.gpsimd.collective_compute(
                    "AllGather", mybir.AluOpType.bypass,
                    replica_groups=[[0, 1, 2, 3], [4, 5, 6, 7]],
                    ins=[kv_in1[:].opt()], outs=[kv_all1[:].opt()])

                # half 2: k m=3..5 + v cols 384:768 -> AG2
                for m in range(3, 6):
                    k_mtile(m)
                nc.sync.dma_start(
                    out=kv_in2[0:KH2].rearrange("(ct p t) -> p ct t", p=128,
                                                t=TLOC),
                    in_=kT[:, 3:6, :])
                v_half(1, v_sb2)
                nc.sync.dma_start(
                    out=kv_in2[KH2:KH2 + VH2].rearrange(
                        "(tt p c) -> p tt c", p=128, c=HC),
                    in_=v_sb2)
                nc.gpsimd.collective_compute(
                    "AllGather", mybir.AluOpType.bypass,
                    replica_groups=[[0, 1, 2, 3], [4, 5, 6, 7]],
                    ins=[kv_in2[:].opt()], outs=[kv_all2[:].opt()])

                # q (pair layout: m-tile = head pair)
                for m in range(6):
                    ps = qkp.tile([128, TLOC], F32, tag="qk")
                    for k in range(6):
                        nc.tensor.matmul(
                            ps, lhsT=wq_sb[:, k, m * 128:(m + 1) * 128],
                            rhs=hT[:, k, :], start=(k == 0), stop=(k == 5))
                    if add_qk_bias:
                        nc.scalar.activation(
                            out=qT[:, m, :], in_=ps,
                            func=mybir.ActivationFunctionType.Copy,
                            bias=bqk_sb[:, 0, m:m + 1])
                    else:
                        nc.scalar.copy(qT[:, m, :], ps)

                # deferred weight/mask DMAs (needed later than x/wk/wq/wv)
                nc.sync.dma_start(out=masks_sb, in_=masks_ext.ap())
                nc.sync.dma_start(
                    out=wp_sb, in_=wp_ext.ap().rearrange("h p c -> p h c"))
                nc.sync.dma_start(
                    out=bfc_sb,
                    in_=bfc_ext.ap().rearrange("(m p) -> p m", p=128))

            # ---------------- attention ----------------
            with tc.tile_pool(name="kch", bufs=1) as kchp, \
                 tc.tile_pool(name="vaug", bufs=1) as vaugp, \
                 tc.tile_pool(name="esb", bufs=4) as esbp, \
                 tc.tile_pool(name="ep", bufs=2, space="PSUM") as epp, \
                 tc.tile_pool(name="avp", bufs=4, space="PSUM") as avpp:

                k_ch = kchp.tile([128, 4, 6, TLOC], BF16)
                v_aug = vaugp.tile([128, NKT, 12 * 65], BF16)
                va4 = v_aug[:].rearrange("p kt (h e) -> p kt h e", e=65)
                nc.vector.memset(va4[:, :, :, 64:65], 1.0)

                for hh, kv_a in enumerate((kv_all1, kv_all2)):
                    for r in range(4):
                        nc.sync.dma_start(
                            out=k_ch[:, r, 3 * hh:3 * hh + 3, :],
                            in_=kv_a[r, 0:KH2].rearrange(
                                "(ct p t) -> p ct t", p=128, t=TLOC))
                    for r in range(4):
                        for tt in range(4):
                            off = KH2 + tt * 128 * HC
                            src = kv_a[r, off:off + 128 * HC].rearrange(
                                "(p h e) -> p h e", p=128, h=6, e=64)
                            kt = (2 * r, 2 * r + 1, 14 - 2 * r,
                                  15 - 2 * r)[tt]
                            nc.gpsimd.dma_start(
                                out=va4[:, kt, 6 * hh:6 * hh + 6, 0:64],
                                in_=src)

                def k_lhsT(kt, p, half):
                    ck = kt // 2
                    r = ck if ck < 4 else 7 - ck
                    loc = (0 if ck < 4 else 256) + (kt % 2) * 128
                    return k_ch[64 * half:64 * half + 64, r, p,
                                loc:loc + 128]

                def finalize_head(h, pav):
                    h2 = h % 2
                    p = h // 2
                    nc.vector.tensor_copy(d_sb[0:1, h2, :], pav[64:65, :])
                    pb = epp.tile([64, TLOC], F32, tag="e", name="pbc")
                    nc.tensor.matmul(pb, lhsT=ones_pad, rhs=d_sb[:, h2, :],
                                     start=True, stop=True)
                    b_sb = small.tile([64, TLOC], F32, tag="bsb", name="bsb")
                    nc.vector.reciprocal_approx_fast(out=b_sb, in_=pb)
                    nc.vector.tensor_mul(yT[64 * h2:64 * h2 + 64, p, :],
                                         pav[0:64, :], b_sb)

                def emit_av(pend):
                    p, e_sb, g = pend
                    for h2 in range(2):
                        h = 2 * p + h2
                        pav = pavs[h]
                        if len(g) == 1:
                            kt = g[0]
                            nc.tensor.matmul(
                                pav,
                                lhsT=v_aug[:, kt, h * 65:(h + 1) * 65],
                                rhs=e_sb[:, h2 * 512:h2 * 512 + 512],
                                start=(kt == 0), stop=(kt == NKT - 1),
                                skip_group_check=True)
                        else:
                            for i, kt in enumerate(g):
                                so = h2 * 512 + i * 256
                                nc.tensor.matmul(
                                    pav[:, 256:512],
                                    lhsT=v_aug[:, kt, h * 65:(h + 1) * 65],
                                    rhs=e_sb[:, so:so + 256],
                                    start=False, stop=(kt == NKT - 1),
                                    skip_group_check=True)
                    if g[-1] == NKT - 1:
                        for h2 in range(2):
                            h = 2 * p + h2
                            finalize_head(h, pavs[h])
                            del pavs[h]

                pavs = {}
                pends = []
                for p in range(6):
                    for h2 in range(2):
                        pavs[2 * p + h2] = avpp.tile(
                            [65, TLOC], F32, tag="av", name=f"pav{2 * p + h2}")
                    for g in GROUPS2:
                        pe = epp.tile([128, 1024], F32, tag="e")
                        if len(g) == 1:
                            kt = g[0]
                            for half in range(2):
                                nc.tensor.matmul(
                                    pe[:, half * 512:half * 512 + 512],
                                    lhsT=k_lhsT(kt, p, half),
                                    rhs=qT[64 * half:64 * half + 64, p, :],
                                    start=True, stop=True)
                            moff = kt * 512
                        else:
                            for half in range(2):
                                for i, kt in enumerate(g):
                                    so = half * 512 + i * 256
                                    nc.tensor.matmul(
                                        pe[:, so:so + 256],
                                        lhsT=k_lhsT(kt, p, half),
                                        rhs=qT[64 * half:64 * half + 64, p,
                                               256:512],
                                        start=True, stop=True)
                            moff = 4096 + (g[0] - 8) * 256
                        e_sb = esbp.tile([128, 1024], BF16, tag="esb")
                        nc.scalar.activation(
                            out=e_sb, in_=pe,
                            func=mybir.ActivationFunctionType.Exp)
                        ms = masks_sb[:, moff:moff + 512]
                        mb = bass.AP(tensor=ms.tensor, offset=ms.offset,
                                     ap=[ms.ap[0], [0, 2], ms.ap[1]])
                        e3 = e_sb[:].rearrange("p (a c) -> p a c", a=2)
                        nc.vector.tensor_mul(e3, e3, mb)
                        pends.append((p, e_sb, g))
                        if len(pends) > 3:
                            emit_av(pends.pop(0))
                for pend in pends:
                    emit_av(pend)
                pends = []

            # ---------------- proj + residual + LN2 ----------------
            with tc.tile_pool(name="pp", bufs=2, space="PSUM") as ppp, \
                 tc.tile_pool(name="ln2", bufs=3) as ln2p, \
                 tc.tile_pool(name="tp2", bufs=2, space="PSUM") as tpp2:

                xn2s = []
                for t in range(4):
                    pp = ppp.tile([128, C], F32, tag="pp")
                    for p in range(6):
                        y_ap = yT[:, p, t * 128:(t + 1) * 128]
                        nc.tensor.matmul(pp[:, 0:512], lhsT=y_ap,
                                         rhs=wp_sb[:, p, 0:512],
                                         start=(p == 0), stop=(p == 5))
                        nc.tensor.matmul(pp[:, 512:768], lhsT=y_ap,
                                         rhs=wp_sb[:, p, 512:768],
                                         start=(p == 0), stop=(p == 5))
                    nc.vector.tensor_add(x_sb[:, t, :], x_sb[:, t, :], pp)
                    if add_proj_bias:
                        nc.vector.tensor_add(x_sb[:, t, :], x_sb[:, t, :],
                                             bout_sb[:, 0, :])
                    xn2 = ln2p.tile([128, C], BF16, tag="xn2", name="xn2")
                    layernorm_to(ln2p, x_sb[:, t, :], xn2, "2")
                    xn2s.append(xn2)
                for t in range(4):
                    for ct in range(6):
                        pt = tpp2.tile([128, 128], BF16, tag="tp2")
                        nc.tensor.transpose(
                            pt, xn2s[t][:, ct * 128:(ct + 1) * 128], ident)
                        nc.vector.tensor_copy(
                            hT[:, ct, t * 128:(t + 1) * 128], pt)

            # ---------------- MLP ----------------
            with tc.tile_pool(name="mlp", bufs=1) as mlpp, \
                 tc.tile_pool(name="wfc", bufs=6) as wfcp, \
                 tc.tile_pool(name="wfc2", bufs=6) as wfc2p, \
                 tc.tile_pool(name="osb", bufs=3) as osbp:

                gT = mlpp.tile([128, 24, TLOC], BF16)
                wfc_t = wfc_ext.ap().rearrange("(k p) n -> p k n", p=128)
                with tc.tile_pool(name="fcp", bufs=2, space="PSUM") as fcpp:
                    for m in range(24):
                        wt = wfcp.tile([128, 6, 128], BF16, tag="wfc")
                        nc.sync.dma_start(
                            out=wt, in_=wfc_t[:, :, m * 128:(m + 1) * 128])
                        pf = fcpp.tile([128, TLOC], F32, tag="fc")
                        for k in range(6):
                            nc.tensor.matmul(pf, lhsT=wt[:, k, :],
                                             rhs=hT[:, k, :],
                                             start=(k == 0), stop=(k == 5))
                        nc.scalar.activation(
                            out=gT[:, m, :], in_=pf,
                            func=mybir.ActivationFunctionType.Gelu_apprx_tanh,
                            bias=bfc_sb[:, m:m + 1])

                wfc2_t = wfc2_ext.ap().rearrange("(k p) n -> k p n", p=128)
                with tc.tile_pool(name="f2p", bufs=1, space="PSUM") as f2pp:
                    pf2s = [f2pp.tile([128, C], F32, tag=f"f2_{t}",
                                      name=f"pf2_{t}")
                            for t in range(4)]
                    for k in range(24):
                        wt2 = wfc2p.tile([128, C], BF16, tag="wfc2")
                        nc.sync.dma_start(out=wt2, in_=wfc2_t[k])
                        for t in range(4):
                            nc.tensor.matmul(
                                pf2s[t][:, 0:512],
                                lhsT=gT[:, k, t * 128:(t + 1) * 128],
                                rhs=wt2[:, 0:512],
                                start=(k == 0), stop=(k == 23))
                            nc.tensor.matmul(
                                pf2s[t][:, 512:768],
                                lhsT=gT[:, k, t * 128:(t + 1) * 128],
                                rhs=wt2[:, 512:768],
                                start=(k == 0), stop=(k == 23))
                    for t in range(4):
                        o_sb = osbp.tile([128, C], F32, tag="osb", name="osb")
                        nc.vector.tensor_add(o_sb, x_sb[:, t, :], pf2s[t])
                        if add_fc2_bias:
                            nc.vector.tensor_add(o_sb, o_sb, bout_sb[:, 1, :])
                        nc.sync.dma_start(
                            out=out_ext[t * 128:(t + 1) * 128, :], in_=o_sb)

    nc.compile()
    return nc


def _preprocess(inputs):
    f = lambda k: np.asarray(inputs[k], np.float32)
    x = f("x"); w_attn = f("w_attn"); b_attn = f("b_attn")
    w_proj = f("w_proj"); b_proj = f("b_proj")
    w_fc = f("w_fc"); b_fc = f("b_fc"); w_fc2 = f("w_fc2"); b_fc2 = f("b_fc2")
    ln1_g = f("ln1_g"); ln1_b = f("ln1_b"); ln2_g = f("ln2_g"); ln2_b = f("ln2_b")

    w_attn_eff = ln1_g[:, None] * w_attn
    b_attn_eff = b_attn + ln1_b @ w_attn
    s = 1.0 / np.sqrt(HD)
    w_q = w_attn_eff[:, 0:C] * s
    w_k = w_attn_eff[:, C:2 * C]
    w_v = w_attn_eff[:, 2 * C:3 * C]
    b_q = b_attn_eff[0:C] * s
    b_k = b_attn_eff[C:2 * C]
    b_v = b_attn_eff[2 * C:3 * C]
    b_proj_eff = b_proj + b_v @ w_proj
    w_fc_eff = ln2_g[:, None] * w_fc
    b_fc_eff = b_fc + ln2_b @ w_fc

    wq16 = np.ascontiguousarray(w_q.astype(BF))
    wk16 = np.ascontiguousarray(w_k.astype(BF))
    wv16 = np.ascontiguousarray(w_v.astype(BF))
    wp16 = np.ascontiguousarray(w_proj.reshape(6, 128, C).astype(BF))
    wfc16 = np.ascontiguousarray(w_fc_eff.astype(BF))
    wfc216 = np.ascontiguousarray(w_fc2.astype(BF))

    bqk = np.stack([b_q, b_k]).astype(np.float32)
    bout = np.stack([b_proj_eff, b_fc2]).astype(np.float32)

    flags = (bool(np.any(bqk != 0)), bool(np.any(b_proj_eff != 0)),
             bool(np.any(b_fc2 != 0)))

    # mask slab [128, 6144] per core-group position j
    kpos = np.arange(128)
    qpos = np.arange(CHUNK)
    masks = np.zeros((4, 128, MASK_W), np.float32)
    for j in range(4):
        for kt in range(NKT):
            gk = kt * 128 + kpos[:, None]
            if kt < 8:
                off = kt * 512
                gq0 = j * CHUNK + qpos[None, :]
                gq1 = (7 - j) * CHUNK + qpos[None, :]
                masks[j, :, off:off + 256] = (gq0 >= gk)
                masks[j, :, off + 256:off + 512] = (gq1 >= gk)
            else:
                off = 4096 + (kt - 8) * 256
                gq1 = (7 - j) * CHUNK + qpos[None, :]
                masks[j, :, off:off + 256] = (gq1 >= gk)
    masks16 = masks.astype(BF)

    in_maps = []
    for c in range(NCORES):
        b, j = c // 4, c % 4
        x_loc = np.concatenate(
            [x[b, j * CHUNK:(j + 1) * CHUNK],
             x[b, (7 - j) * CHUNK:(8 - j) * CHUNK]]).astype(np.float32)
        in_maps.append({
            "x": np.ascontiguousarray(x_loc),
            "wq": wq16, "wk": wk16, "wv": wv16, "wp": wp16,
            "wfc": wfc16, "wfc2": wfc216,
            "masks": np.ascontiguousarray(masks16[j]),
            "bqk": bqk, "bfc": b_fc_eff.astype(np.float32), "bout": bout,
        })
    return in_maps, flags


def kernel(**inputs):
    global LAST_EXEC_NS, LAST_RESULTS
    in_maps, flags = _preprocess(inputs)
    if flags not in _CACHE:
        _CACHE[flags] = _build(*flags)
    nc = _CACHE[flags]
    trace = bool(os.environ.get("BASS_KERNEL_TRACE"))
    res = run_bass_kernel_spmd(nc, in_maps, core_ids=list(range(NCORES)),
                               trace=trace)
    LAST_EXEC_NS = res.exec_time_ns
    LAST_RESULTS = res
    out = np.empty((B, T, C), np.float32)
    for c in range(NCORES):
        b, j = c // 4, c % 4
        o = res.results[c]["out"]
        out[b, j * CHUNK:(j + 1) * CHUNK] = o[0:CHUNK]
        out[b, (7 - j) * CHUNK:(8 - j) * CHUNK] = o[CHUNK:TLOC]
    return out


# revision 4
# speedup vs baseline: 1.1399x; 1.1399x over previous
"""GPT-2 style transformer block on 8 TRN2 NeuronCores.

Sharding: token-data-parallel. Each batch's 2048 tokens are split into 8
chunks of 256; core c owns batch c//4 and chunks {j, 7-j} (j = c%4) so
causal attention work is balanced. QKV/proj/MLP/LN are purely local; the
only collectives are two AllGathers (k^T+v combined, in two halves)
within each 4-core batch group, bf16 on the wire so the gathered data is
DMA'd straight into matmul layouts (no cast pass).

Score matmuls are head-PAIR packed: heads 2p/2p+1 run as two concurrent
row-tiled K=64 matmuls (tile_position row groups 0/64) writing the two
banks of one [128,1024] PSUM tile; one exp covers both heads, and the
causal 0/1 mask is applied with a stride-0-broadcast tensor_tensor so a
single 512-wide mask slab serves both heads. proj is pair-packed too
(K=128 = two heads' hd rows). The v-bias is folded into the proj bias
via softmax-rows-sum-to-one; the attention 1/sqrt(hd) scale is folded
into w_q. Softmax runs without max-subtraction (scores are O(1));
denominators come free as a 65th ones-column in the av matmul.
"""

import os
import sys

sys.path.insert(0, "/opt/trn_rl_repo")

import numpy as np
import ml_dtypes

import concourse.bass as bass
import concourse.tile as tile
from concourse import bacc, mybir
from concourse.bass_utils import run_bass_kernel_spmd
from concourse.masks import make_identity

F32 = mybir.dt.float32
FP8 = mybir.dt.float8e4
BF16 = mybir.dt.bfloat16
BF = ml_dtypes.bfloat16

B, T, C, H, HD = 2, 2048, 768, 12, 64
EPS = 1e-5
NCORES = 8
CHUNK = 256            # global chunk size (tokens)
TLOC = 512             # local tokens per core (2 chunks)
NKT = T // 128         # 16 key tiles per batch
MASK_W = 8 * 512 + 8 * 256   # 6144

KH2 = 3 * 128 * TLOC   # k^T half: [3 ct][128 p][512 t]
VH2 = 4 * 128 * 384    # v half:   [4 tt][128 p][384 c]
HC = 384

# exp groups per head-pair: kt<8 -> one kt per [128,1024] psum
# (both heads x 512); kt>=8 -> two kt per psum (both heads x 2x256).
GROUPS2 = [(kt,) for kt in range(8)] + [(8, 9), (10, 11), (12, 13), (14, 15)]

LAST_EXEC_NS = None
LAST_RESULTS = None
_CACHE = {}


def _build(add_qk_bias, add_proj_bias, add_fc2_bias):
    nc = bacc.Bacc("TRN2", target_bir_lowering=False, debug=False,
                   num_devices=NCORES)

    x_ext = nc.dram_tensor("x", [TLOC, C], F32, kind="ExternalInput")
    wq_ext = nc.dram_tensor("wq", [C, C], BF16, kind="ExternalInput")
    wk_ext = nc.dram_tensor("wk", [C, C], BF16, kind="ExternalInput")
    wv_ext = nc.dram_tensor("wv", [C, C], BF16, kind="ExternalInput")
    wp_ext = nc.dram_tensor("wp", [6, 128, C], BF16, kind="ExternalInput")
    wfc_ext = nc.dram_tensor("wfc", [C, 4 * C], BF16, kind="ExternalInput")
    wfc2_ext = nc.dram_tensor("wfc2", [4 * C, C], BF16, kind="ExternalInput")
    masks_ext = nc.dram_tensor("masks", [128, MASK_W], BF16,
                               kind="ExternalInput")
    bqk_ext = nc.dram_tensor("bqk", [2, C], F32, kind="ExternalInput")
    bfc_ext = nc.dram_tensor("bfc", [4 * C], F32, kind="ExternalInput")
    bout_ext = nc.dram_tensor("bout", [2, C], F32, kind="ExternalInput")
    out_ext = nc.dram_tensor("out", [TLOC, C], F32, kind="ExternalOutput")

    with tile.TileContext(nc) as tc:
        with tc.tile_pool(name="dram", bufs=1, space="DRAM") as dram, \
             tc.tile_pool(name="singles", bufs=1) as singles, \
             tc.tile_pool(name="persist", bufs=1) as persist, \
             tc.tile_pool(name="small", bufs=3) as small:

            kv_in1 = dram.tile([KH2 + VH2], FP8)
            kv_all1 = dram.tile([4, KH2 + VH2], FP8)
            kv_in2 = dram.tile([KH2 + VH2], FP8)
            kv_all2 = dram.tile([4, KH2 + VH2], FP8)

            # x first: LN1 needs it before anything else
            x_sb = persist.tile([128, 4, C], F32)     # local x, becomes xmid
            for t in range(4):
                nc.sync.dma_start(out=x_sb[:, t, :],
                                  in_=x_ext[t * 128:(t + 1) * 128, :])

            ident = singles.tile([128, 128], BF16)
            make_identity(nc, ident)
            eps_sb = singles.tile([128, 1], F32)
            nc.vector.memset(eps_sb, EPS)
            ones_pad = singles.tile([128, 64], F32)
            nc.vector.memset(ones_pad, 0.0)
            nc.vector.memset(ones_pad[0:1, :], 1.0)
            d_sb = singles.tile([128, 2, TLOC], F32)
            nc.vector.memset(d_sb, 1.0)

            bqk_sb = singles.tile([128, 2, 6], F32)
            if add_qk_bias:
                nc.sync.dma_start(
                    out=bqk_sb,
                    in_=bqk_ext.ap().rearrange("b (m p) -> p b m", p=128))
            bout_sb = singles.tile([128, 2, C], F32)
            if add_proj_bias or add_fc2_bias:
                bc = bout_ext.ap()
                nc.sync.dma_start(
                    out=bout_sb,
                    in_=bass.AP(tensor=bc.tensor, offset=bc.offset,
                                ap=[[0, 128], bc.ap[0], bc.ap[1]]))

            masks_sb = persist.tile([128, MASK_W], BF16)
            hT = persist.tile([128, 6, TLOC], BF16)   # h^T, reused for h2^T
            qT = persist.tile([128, 6, TLOC], BF16)   # head-pair layout
            yT = persist.tile([128, 6, TLOC], BF16)   # head-pair layout
            wp_sb = persist.tile([128, 6, C], BF16)
            bfc_sb = singles.tile([128, 24], F32)

            def layernorm_to(pool, xt, dst, tagsuf):
                stats = pool.tile([128, 3, 6], F32, tag="st" + tagsuf,
                                  name="st" + tagsuf)
                for sg in range(3):
                    nc.vector.bn_stats(out=stats[:, sg, :],
                                       in_=xt[:, sg * 256:(sg + 1) * 256])
                mv = pool.tile([128, 2], F32, tag="mv" + tagsuf,
                               name="mv" + tagsuf)
                nc.vector.bn_aggr(out=mv, in_=stats)
                nc.scalar.activation(out=mv[:, 1:2], in_=mv[:, 1:2],
                                     func=mybir.ActivationFunctionType.Sqrt,
                                     bias=eps_sb)
                nc.vector.reciprocal(out=mv[:, 1:2], in_=mv[:, 1:2])
                nc.vector.tensor_scalar(out=dst, in0=xt,
                                        scalar1=mv[:, 0:1], scalar2=mv[:, 1:2],
                                        op0=mybir.AluOpType.subtract,
                                        op1=mybir.AluOpType.mult)

            # ---------------- LN1 + transpose + QKV + AGs ----------------
            with tc.tile_pool(name="ln", bufs=3) as lnp, \
                 tc.tile_pool(name="tp", bufs=2, space="PSUM") as tpp, \
                 tc.tile_pool(name="qkp", bufs=2, space="PSUM") as qkp, \
                 tc.tile_pool(name="vp", bufs=2, space="PSUM") as vpp, \
                 tc.tile_pool(name="vsb", bufs=1) as vsbp:

                kT = vsbp.tile([128, 6, TLOC], FP8)
                v_sb1 = vsbp.tile([128, 4, HC], FP8)
                v_sb2 = vsbp.tile([128, 4, HC], FP8)
                wk_sb = vsbp.tile([128, 6, C], BF16)
                wq_sb = vsbp.tile([128, 6, C], BF16)
                wv_sb = vsbp.tile([128, 6, C], BF16)
                for sb, ext in ((wk_sb, wk_ext), (wv_sb, wv_ext),
                                (wq_sb, wq_ext)):
                    nc.sync.dma_start(
                        out=sb,
                        in_=ext.ap().rearrange("(ct p) c -> p ct c", p=128))

                for t in range(4):
                    xn = lnp.tile([128, C], BF16, tag="xn")
                    layernorm_to(lnp, x_sb[:, t, :], xn, "1")
                    for ct in range(6):
                        pt = tpp.tile([128, 128], BF16, tag="tp")
                        nc.tensor.transpose(
                            pt, xn[:, ct * 128:(ct + 1) * 128], ident)
                        nc.scalar.copy(hT[:, ct, t * 128:(t + 1) * 128], pt)

                def k_mtile(m):
                    ps = qkp.tile([128, TLOC], F32, tag="qk", name="psk")
                    for k in range(6):
                        nc.tensor.matmul(
                            ps, lhsT=wk_sb[:, k, m * 128:(m + 1) * 128],
                            rhs=hT[:, k, :], start=(k == 0), stop=(k == 5))
                    if add_qk_bias:
                        nc.scalar.activation(
                            out=kT[:, m, :], in_=ps,
                            func=mybir.ActivationFunctionType.Copy,
                            bias=bqk_sb[:, 1, m:m + 1])
                    else:
                        nc.scalar.copy(kT[:, m, :], ps)

                def v_half(half, v_sb):
                    c0 = half * HC
                    for tt in range(4):
                        pv = vpp.tile([128, HC], F32, tag="v")
                        for k in range(6):
                            nc.tensor.matmul(
                                pv, lhsT=hT[:, k, tt * 128:(tt + 1) * 128],
                                rhs=wv_sb[:, k, c0:c0 + HC],
                                start=(k == 0), stop=(k == 5))
                        nc.scalar.copy(v_sb[:, tt, :], pv)

                # half 1: k m=0..2 + v cols 0:384 -> AG1
                for m in range(3):
                    k_mtile(m)
                nc.sync.dma_start(
                    out=kv_in1[0:KH2].rearrange("(ct p t) -> p ct t", p=128,
                                                t=TLOC),
                    in_=kT[:, 0:3, :])
                v_half(0, v_sb1)
                nc.sync.dma_start(
                    out=kv_in1[KH2:KH2 + VH2].rearrange(
                        "(tt p c) -> p tt c", p=128, c=HC),
                    in_=v_sb1)
                nc.gpsimd.collective_compute(
                    "AllGather", mybir.AluOpType.bypass,
                    replica_groups=[[0, 1, 2, 3], [4, 5, 6, 7]],
                    ins=[kv_in1[:].opt()], outs=[kv_all1[:].opt()])

                # half 2: k m=3..5 + v cols 384:768 -> AG2
                for m in range(3, 6):
                    k_mtile(m)
                nc.sync.dma_start(
                    out=kv_in2[0:KH2].rearrange("(ct p t) -> p ct t", p=128,
                                                t=TLOC),
                    in_=kT[:, 3:6, :])
                v_half(1, v_sb2)
                nc.sync.dma_start(
                    out=kv_in2[KH2:KH2 + VH2].rearrange(
                        "(tt p c) -> p tt c", p=128, c=HC),
                    in_=v_sb2)
                nc.gpsimd.collective_compute(
                    "AllGather", mybir.AluOpType.bypass,
                    replica_groups=[[0, 1, 2, 3], [4, 5, 6, 7]],
                    ins=[kv_in2[:].opt()], outs=[kv_all2[:].opt()])

                # q (pair layout: m-tile = head pair)
                for m in range(6):
                    ps = qkp.tile([128, TLOC], F32, tag="qk")
                    for k in range(6):
                        nc.tensor.matmul(
                            ps, lhsT=wq_sb[:, k, m * 128:(m + 1) * 128],
                            rhs=hT[:, k, :], start=(k == 0), stop=(k == 5))
                    if add_qk_bias:
                        nc.scalar.activation(
                            out=qT[:, m, :], in_=ps,
                            func=mybir.ActivationFunctionType.Copy,
                            bias=bqk_sb[:, 0, m:m + 1])
                    else:
                        nc.scalar.copy(qT[:, m, :], ps)

                # deferred weight/mask DMAs (needed later than x/wk/wq/wv)
                nc.sync.dma_start(out=masks_sb, in_=masks_ext.ap())
                nc.sync.dma_start(
                    out=wp_sb, in_=wp_ext.ap().rearrange("h p c -> p h c"))
                nc.sync.dma_start(
                    out=bfc_sb,
                    in_=bfc_ext.ap().rearrange("(m p) -> p m", p=128))

            # ---------------- attention ----------------
            with tc.tile_pool(name="kch", bufs=1) as kchp, \
                 tc.tile_pool(name="vaug", bufs=1) as vaugp, \
                 tc.tile_pool(name="esb", bufs=4) as esbp, \
                 tc.tile_pool(name="ep", bufs=2, space="PSUM") as epp, \
                 tc.tile_pool(name="avp", bufs=4, space="PSUM") as avpp:

                k_ch = kchp.tile([128, 4, 6, TLOC], FP8)
                v_aug = vaugp.tile([128, NKT, 12 * 65], FP8)
                va4 = v_aug[:].rearrange("p kt (h e) -> p kt h e", e=65)
                nc.vector.memset(va4[:, :, :, 64:65], 1.0)

                for hh, kv_a in enumerate((kv_all1, kv_all2)):
                    for r in range(4):
                        nc.sync.dma_start(
                            out=k_ch[:, r, 3 * hh:3 * hh + 3, :],
                            in_=kv_a[r, 0:KH2].rearrange(
                                "(ct p t) -> p ct t", p=128, t=TLOC))
                    for r in range(4):
                        for tt in range(4):
                            off = KH2 + tt * 128 * HC
                            src = kv_a[r, off:off + 128 * HC].rearrange(
                                "(p h e) -> p h e", p=128, h=6, e=64)
                            kt = (2 * r, 2 * r + 1, 14 - 2 * r,
                                  15 - 2 * r)[tt]
                            nc.gpsimd.dma_start(
                                out=va4[:, kt, 6 * hh:6 * hh + 6, 0:64],
                                in_=src)

                def k_lhsT(kt, p, half):
                    ck = kt // 2
                    r = ck if ck < 4 else 7 - ck
                    loc = (0 if ck < 4 else 256) + (kt % 2) * 128
                    return k_ch[64 * half:64 * half + 64, r, p,
                                loc:loc + 128]

                def finalize_head(h, pav):
                    h2 = h % 2
                    p = h // 2
                    nc.vector.tensor_copy(d_sb[0:1, h2, :], pav[64:65, :])
                    pb = epp.tile([64, TLOC], F32, tag="e", name="pbc")
                    nc.tensor.matmul(pb, lhsT=ones_pad, rhs=d_sb[:, h2, :],
                                     start=True, stop=True)
                    b_sb = small.tile([64, TLOC], F32, tag="bsb", name="bsb")
                    nc.vector.reciprocal_approx_fast(out=b_sb, in_=pb)
                    nc.vector.tensor_mul(yT[64 * h2:64 * h2 + 64, p, :],
                                         pav[0:64, :], b_sb)

                def emit_av(pend):
                    p, e_sb, g = pend
                    for h2 in range(2):
                        h = 2 * p + h2
                        pav = pavs[h]
                        if len(g) == 1:
                            kt = g[0]
                            nc.tensor.matmul(
                                pav,
                                lhsT=v_aug[:, kt, h * 65:(h + 1) * 65],
                                rhs=e_sb[:, h2 * 512:h2 * 512 + 512],
                                start=(kt == 0), stop=(kt == NKT - 1),
                                skip_group_check=True)
                        else:
                            for i, kt in enumerate(g):
                                so = h2 * 512 + i * 256
                                nc.tensor.matmul(
                                    pav[:, 256:512],
                                    lhsT=v_aug[:, kt, h * 65:(h + 1) * 65],
                                    rhs=e_sb[:, so:so + 256],
                                    start=False, stop=(kt == NKT - 1),
                                    skip_group_check=True)
                    if g[-1] == NKT - 1:
                        for h2 in range(2):
                            h = 2 * p + h2
                            finalize_head(h, pavs[h])
                            del pavs[h]

                pavs = {}
                pends = []
                for p in range(6):
                    for h2 in range(2):
                        pavs[2 * p + h2] = avpp.tile(
                            [65, TLOC], F32, tag="av", name=f"pav{2 * p + h2}")
                    for g in GROUPS2:
                        pe = epp.tile([128, 1024], F32, tag="e")
                        if len(g) == 1:
                            kt = g[0]
                            for half in range(2):
                                nc.tensor.matmul(
                                    pe[:, half * 512:half * 512 + 512],
                                    lhsT=k_lhsT(kt, p, half),
                                    rhs=qT[64 * half:64 * half + 64, p, :],
                                    start=True, stop=True)
                            moff = kt * 512
                        else:
                            for half in range(2):
                                for i, kt in enumerate(g):
                                    so = half * 512 + i * 256
                                    nc.tensor.matmul(
                                        pe[:, so:so + 256],
                                        lhsT=k_lhsT(kt, p, half),
                                        rhs=qT[64 * half:64 * half + 64, p,
                                               256:512],
                                        start=True, stop=True)
                            moff = 4096 + (g[0] - 8) * 256
                        e_sb = esbp.tile([128, 1024], BF16, tag="esb")
                        nc.scalar.activation(
                            out=e_sb, in_=pe,
                            func=mybir.ActivationFunctionType.Exp)
                        ms = masks_sb[:, moff:moff + 512]
                        mb = bass.AP(tensor=ms.tensor, offset=ms.offset,
                                     ap=[ms.ap[0], [0, 2], ms.ap[1]])
                        e3 = e_sb[:].rearrange("p (a c) -> p a c", a=2)
                        nc.vector.tensor_mul(e3, e3, mb)
                        pends.append((p, e_sb, g))
                        if len(pends) > 3:
                            emit_av(pends.pop(0))
                for pend in pends:
                    emit_av(pend)
                pends = []

            # ---------------- proj + residual + LN2 ----------------
            with tc.tile_pool(name="pp", bufs=2, space="PSUM") as ppp, \
                 tc.tile_pool(name="ln2", bufs=3) as ln2p, \
                 tc.tile_pool(name="tp2", bufs=2, space="PSUM") as tpp2:

                xn2s = []
                for t in range(4):
                    pp = ppp.tile([128, C], F32, tag="pp")
                    for p in range(6):
                        y_ap = yT[:, p, t * 128:(t + 1) * 128]
                        nc.tensor.matmul(pp[:, 0:512], lhsT=y_ap,
                                         rhs=wp_sb[:, p, 0:512],
                                         start=(p == 0), stop=(p == 5))
                        nc.tensor.matmul(pp[:, 512:768], lhsT=y_ap,
                                         rhs=wp_sb[:, p, 512:768],
                                         start=(p == 0), stop=(p == 5))
                    nc.vector.tensor_add(x_sb[:, t, :], x_sb[:, t, :], pp)
                    if add_proj_bias:
                        nc.vector.tensor_add(x_sb[:, t, :], x_sb[:, t, :],
                                             bout_sb[:, 0, :])
                    xn2 = ln2p.tile([128, C], BF16, tag="xn2", name="xn2")
                    layernorm_to(ln2p, x_sb[:, t, :], xn2, "2")
                    xn2s.append(xn2)
                for t in range(4):
                    for ct in range(6):
                        pt = tpp2.tile([128, 128], BF16, tag="tp2")
                        nc.tensor.transpose(
                            pt, xn2s[t][:, ct * 128:(ct + 1) * 128], ident)
                        nc.vector.tensor_copy(
                            hT[:, ct, t * 128:(t + 1) * 128], pt)

            # ---------------- MLP ----------------
            with tc.tile_pool(name="mlp", bufs=1) as mlpp, \
                 tc.tile_pool(name="wfc", bufs=6) as wfcp, \
                 tc.tile_pool(name="wfc2", bufs=6) as wfc2p, \
                 tc.tile_pool(name="osb", bufs=3) as osbp:

                gT = mlpp.tile([128, 24, TLOC], BF16)
                wfc_t = wfc_ext.ap().rearrange("(k p) n -> p k n", p=128)
                with tc.tile_pool(name="fcp", bufs=2, space="PSUM") as fcpp:
                    for m in range(24):
                        wt = wfcp.tile([128, 6, 128], BF16, tag="wfc")
                        nc.sync.dma_start(
                            out=wt, in_=wfc_t[:, :, m * 128:(m + 1) * 128])
                        pf = fcpp.tile([128, TLOC], F32, tag="fc")
                        for k in range(6):
                            nc.tensor.matmul(pf, lhsT=wt[:, k, :],
                                             rhs=hT[:, k, :],
                                             start=(k == 0), stop=(k == 5))
                        nc.scalar.activation(
                            out=gT[:, m, :], in_=pf,
                            func=mybir.ActivationFunctionType.Gelu_apprx_tanh,
                            bias=bfc_sb[:, m:m + 1])

                wfc2_t = wfc2_ext.ap().rearrange("(k p) n -> k p n", p=128)
                with tc.tile_pool(name="f2p", bufs=1, space="PSUM") as f2pp:
                    pf2s = [f2pp.tile([128, C], F32, tag=f"f2_{t}",
                                      name=f"pf2_{t}")
                            for t in range(4)]
                    for k in range(24):
                        wt2 = wfc2p.tile([128, C], BF16, tag="wfc2")
                        nc.sync.dma_start(out=wt2, in_=wfc2_t[k])
                        for t in range(4):
                            nc.tensor.matmul(
                                pf2s[t][:, 0:512],
                                lhsT=gT[:, k, t * 128:(t + 1) * 128],
                                rhs=wt2[:, 0:512],
                                start=(k == 0), stop=(k == 23))
                            nc.tensor.matmul(
                                pf2s[t][:, 512:768],
                                lhsT=gT[:, k, t * 128:(t + 1) * 128],
                                rhs=wt2[:, 512:768],
                                start=(k == 0), stop=(k == 23))
                    for t in range(4):
                        o_sb = osbp.tile([128, C], F32, tag="osb", name="osb")
                        nc.vector.tensor_add(o_sb, x_sb[:, t, :], pf2s[t])
                        if add_fc2_bias:
                            nc.vector.tensor_add(o_sb, o_sb, bout_sb[:, 1, :])
                        nc.sync.dma_start(
                            out=out_ext[t * 128:(t + 1) * 128, :], in_=o_sb)

    nc.compile()
    return nc


def _preprocess(inputs):
    f = lambda k: np.asarray(inputs[k], np.float32)
    x = f("x"); w_attn = f("w_attn"); b_attn = f("b_attn")
    w_proj = f("w_proj"); b_proj = f("b_proj")
    w_fc = f("w_fc"); b_fc = f("b_fc"); w_fc2 = f("w_fc2"); b_fc2 = f("b_fc2")
    ln1_g = f("ln1_g"); ln1_b = f("ln1_b"); ln2_g = f("ln2_g"); ln2_b = f("ln2_b")

    w_attn_eff = ln1_g[:, None] * w_attn
    b_attn_eff = b_attn + ln1_b @ w_attn
    s = 1.0 / np.sqrt(HD)
    w_q = w_attn_eff[:, 0:C] * s
    w_k = w_attn_eff[:, C:2 * C]
    w_v = w_attn_eff[:, 2 * C:3 * C]
    b_q = b_attn_eff[0:C] * s
    b_k = b_attn_eff[C:2 * C]
    b_v = b_attn_eff[2 * C:3 * C]
    b_proj_eff = b_proj + b_v @ w_proj
    w_fc_eff = ln2_g[:, None] * w_fc
    b_fc_eff = b_fc + ln2_b @ w_fc

    wq16 = np.ascontiguousarray(w_q.astype(BF))
    wk16 = np.ascontiguousarray(w_k.astype(BF))
    wv16 = np.ascontiguousarray(w_v.astype(BF))
    wp16 = np.ascontiguousarray(w_proj.reshape(6, 128, C).astype(BF))
    wfc16 = np.ascontiguousarray(w_fc_eff.astype(BF))
    wfc216 = np.ascontiguousarray(w_fc2.astype(BF))

    bqk = np.stack([b_q, b_k]).astype(np.float32)
    bout = np.stack([b_proj_eff, b_fc2]).astype(np.float32)

    flags = (bool(np.any(bqk != 0)), bool(np.any(b_proj_eff != 0)),
             bool(np.any(b_fc2 != 0)))

    # mask slab [128, 6144] per core-group position j
    kpos = np.arange(128)
    qpos = np.arange(CHUNK)
    masks = np.zeros((4, 128, MASK_W), np.float32)
    for j in range(4):
        for kt in range(NKT):
            gk = kt * 128 + kpos[:, None]
            if kt < 8:
                off = kt * 512
                gq0 = j * CHUNK + qpos[None, :]
                gq1 = (7 - j) * CHUNK + qpos[None, :]
                masks[j, :, off:off + 256] = (gq0 >= gk)
                masks[j, :, off + 256:off + 512] = (gq1 >= gk)
            else:
                off = 4096 + (kt - 8) * 256
                gq1 = (7 - j) * CHUNK + qpos[None, :]
                masks[j, :, off:off + 256] = (gq1 >= gk)
    masks16 = masks.astype(BF)

    in_maps = []
    for c in range(NCORES):
        b, j = c // 4, c % 4
        x_loc = np.concatenate(
            [x[b, j * CHUNK:(j + 1) * CHUNK],
             x[b, (7 - j) * CHUNK:(8 - j) * CHUNK]]).astype(np.float32)
        in_maps.append({
            "x": np.ascontiguousarray(x_loc),
            "wq": wq16, "wk": wk16, "wv": wv16, "wp": wp16,
            "wfc": wfc16, "wfc2": wfc216,
            "masks": np.ascontiguousarray(masks16[j]),
            "bqk": bqk, "bfc": b_fc_eff.astype(np.float32), "bout": bout,
        })
    return in_maps, flags


def kernel(**inputs):
    global LAST_EXEC_NS, LAST_RESULTS
    in_maps, flags = _preprocess(inputs)
    if flags not in _CACHE:
        _CACHE[flags] = _build(*flags)
    nc = _CACHE[flags]
    trace = bool(os.environ.get("BASS_KERNEL_TRACE"))
    res = run_bass_kernel_spmd(nc, in_maps, core_ids=list(range(NCORES)),
                               trace=trace)
    LAST_EXEC_NS = res.exec_time_ns
    LAST_RESULTS = res
    out = np.empty((B, T, C), np.float32)
    for c in range(NCORES):
        b, j = c // 4, c % 4
        o = res.results[c]["out"]
        out[b, j * CHUNK:(j + 1) * CHUNK] = o[0:CHUNK]
        out[b, (7 - j) * CHUNK:(8 - j) * CHUNK] = o[CHUNK:TLOC]
    return out
